# revision 34
# baseline (speedup 1.0000x reference)
"""Trainium2 kernel for CrossSiloAggregator (gnn_message_passing).

Reference semantics:
    local_emb = local_embeddings[local_indices]            # [M, D] gather
    w = sigmoid(concat([local_emb, foreign], -1) @ W + b)  # [M, 1]
    updated = w * local_emb + (1 - w) * foreign            # [M, D]
    out = local_embeddings.at[local_indices].set(updated)

Strategy (8 NeuronCores, memory-bound):
  - Host gathers the M=200k boundary rows (general in local_indices),
    shards them evenly across 8 cores (25k rows each) and passes each
    shard TRANSPOSED ([D=128 partitions, rows free]).  The transposed
    layout lets the TensorEngine compute the attention logits as two
    K=128 matmuls (Wl.T @ lT + Wf.T @ fT).
  - All device IO is fp16: per-NC DMA bandwidth is the roofline (~380
    GB/s measured via load-only ablation; R+W share it), so halving the
    bytes halves the floor: 3 x 6.4 MB = 19.2 MB/core ~ 51-57 us.  The
    fp16 rounding (2^-11) keeps rel err ~1e-3, far under the 2e-2 gate.
    Measured on HW: ~71-77us/core vs 56.6us dma-only ablation.
  - Engine busy per core (rows=25000, fp16, tile-sim model):
      SP/DMA 19.2 MB (in 12.8 + out 6.4, one shared pipe)  ~57us <- floor
      DVE    sub/mul/add fp16 (2x_1P packed tensor_tensor) ~40us
      ACT    sigmoid per 512-slice (PSUM->SBUF)            ~31us
      PE     logits, fp16 matmul 1 cyc/row                 ~21-31us
      GPSIMD partition_broadcast of w per chunk            ~21us
  - const loads (wl/wf/bb) go on the ACT HW-DGE ring so their ~us
    completion latency cannot head-block chunk 0's loads on the SP ring
    (-4.5us measured).
  - Measured SLOWER on this HW despite the cost model preferring them
    (tile-sim model diverges from HW for structural changes -- trust
    only within-process K=1-vs-K=51 fast-dispatch differencing):
      int8 output via DVE converting add   (90us)
      chunk 2048 + split 2 + deep buffers  (116us)
      blend ops on gpsimd, tapered chunks, chunked-DRAM layout,
      out-DMA on the ACT ring, out_split=1
  - Device computes only the 200k updated rows; the untouched 800k rows
    are carried to the output by the host-side unshard (a copy the
    full-IO contract requires anyway).
"""

import sys

import numpy as np

for _p in ("/opt/trn_rl_repo", "/root/.axon_site/_ro/trn_rl_repo"):
    if _p not in sys.path:  # harness may run without PYTHONPATH
        sys.path.append(_p)

P = 128          # partitions == embedding dim
N_CORES = 8
N_FOREIGN = 200_000
ROWS_PER_CORE = N_FOREIGN // N_CORES   # 25000
CHUNK = 4096     # rows per SBUF tile
SLICE = 512      # matmul free-dim (one PSUM bank)

# int8 output path: host pre-scales l,f by 1/s (s folded into W_att so the
# logits are unchanged); the blend runs in scaled space where |out| <= 126.5
# and the final add converts straight to int8; host multiplies by s after.
OUT_DTYPE = None   # "i8" path measured slower on HW (DVE convert at 1x)
OUT_HEADROOM = 126.5   # int8 ceiling minus fp16-rounding safety margin


def _chunks(rows, chunk, sizes=None):
    out = []
    off = 0
    if sizes is not None:
        assert sum(sizes) == rows, (sum(sizes), rows)
        for n in sizes:
            out.append((off, n))
            off += n
        return out
    while off < rows:
        n = min(chunk, rows - off)
        out.append((off, n))
        off += n
    return out


def build_nc(rows=ROWS_PER_CORE, chunk=CHUNK, slice_n=SLICE, repeats=1,
             bufs_io=3, bufs_o=3, bufs_w=1, bufs_wb=2, bufs_log=3,
             mul_eng="dve", add_eng="dve", sub_eng="dve", skip=(),
             io_dtype="f16", out_dtype=OUT_DTYPE, split=1, out_eng="sync",
             add_engs=("dve",), mul_engs=("dve",),
             out_split=2, const_eng="act", chunked_dram=False,
             chunk_sizes=None):
    """Build the per-core Bass program (SPMD: identical on all cores).

    repeats>1 re-runs the whole pass over the same DRAM buffers (used by
    the timing harness to difference out fixed dispatch overhead)."""
    from contextlib import ExitStack

    import concourse.bacc as bacc
    import concourse.mybir as mybir
    import concourse.tile as tile

    f32 = mybir.dt.float32
    fio = {"f32": f32, "f16": mybir.dt.float16,
           "bf16": mybir.dt.bfloat16}[io_dtype]
    fout = {None: fio, "i8": mybir.dt.int8, "f16": mybir.dt.float16,
            "f32": f32}[out_dtype]
    nc = bacc.Bacc("TRN2")

    if chunked_dram:
        # [nch, P, chunk]: every chunk load/store is one contiguous block
        assert rows % chunk == 0
        nch = rows // chunk
        lT = nc.dram_tensor("lT", [nch, P, chunk], fio, kind="ExternalInput")
        fT = nc.dram_tensor("fT", [nch, P, chunk], fio, kind="ExternalInput")
        outT = nc.dram_tensor("outT", [nch, P, chunk], fout, kind="ExternalOutput")
    else:
        lT = nc.dram_tensor("lT", [P, rows], fio, kind="ExternalInput")
        fT = nc.dram_tensor("fT", [P, rows], fio, kind="ExternalInput")
        outT = nc.dram_tensor("outT", [P, rows], fout, kind="ExternalOutput")
    wl = nc.dram_tensor("wl", [P, 1], fio, kind="ExternalInput")
    wf = nc.dram_tensor("wf", [P, 1], fio, kind="ExternalInput")
    bb = nc.dram_tensor("bb", [1, 1], f32, kind="ExternalInput")

    def dram_sl(t, off, c0, h):
        if chunked_dram:
            return t[off // chunk, :, c0 : c0 + h]
        return t[:, off + c0 : off + c0 + h]

    def eng(name):
        return {"dve": nc.vector, "gpsimd": nc.gpsimd}[name]

    # out-DMA on a different HW-DGE ring (qActDynamicHW) so its sequencer
    # wait can't block the issue of later chunks' input loads on qSPDynamicHW
    out_dma = {"sync": nc.sync, "act": nc.scalar,
               "dve": nc.vector, "gpsimd": nc.gpsimd}[out_eng]

    with tile.TileContext(nc) as tc, ExitStack() as ctx:
        consts = ctx.enter_context(tc.tile_pool(name="consts", bufs=1))
        io_l = ctx.enter_context(tc.tile_pool(name="io_l", bufs=bufs_io))
        io_f = ctx.enter_context(tc.tile_pool(name="io_f", bufs=bufs_io))
        io_o = ctx.enter_context(tc.tile_pool(name="io_o", bufs=bufs_o))
        wpool = ctx.enter_context(tc.tile_pool(name="wpool", bufs=bufs_w))
        wbpool = ctx.enter_context(tc.tile_pool(name="wbpool", bufs=bufs_wb))
        ps_log = ctx.enter_context(
            tc.tile_pool(name="ps_log", bufs=bufs_log, space="PSUM"))

        # consts on the ACT HW-DGE ring: 3 tiny loads with ~us completion
        # latency each must not head-block the first chunk loads on SP's ring
        cdma = {"sync": nc.sync, "act": nc.scalar}[const_eng]
        wl_sb = consts.tile([P, 1], fio)
        cdma.dma_start(out=wl_sb, in_=wl[:])
        wf_sb = consts.tile([P, 1], fio)
        cdma.dma_start(out=wf_sb, in_=wf[:])
        b_sb = consts.tile([1, 1], f32)
        cdma.dma_start(out=b_sb, in_=bb[:])

        for off, n in _chunks(rows, chunk, chunk_sizes) * repeats:
            nsl = (n + slice_n - 1) // slice_n

            l_t = io_l.tile([P, n], fio, tag="l")
            f_t = io_f.tile([P, n], fio, tag="f")
            o_t = io_o.tile([P, n], fio, tag="o")
            # final add converts to the (possibly narrower) out dtype
            o2_t = o_t if fout == fio else io_o.tile([P, n], fout, tag="o8")
            w_sb = wpool.tile([1, n], fio, tag="w")
            wb_t = wbpool.tile([P, n], fio, tag="wb")
            nc.sync.dma_start(out=l_t, in_=dram_sl(lT, off, 0, n))
            nc.sync.dma_start(out=f_t, in_=dram_sl(fT, off, 0, n))

            # o = l - f (chunk-wide)
            if "sub" not in skip:
                eng(sub_eng).tensor_sub(out=o_t, in0=l_t, in1=f_t)

            for s in range(nsl):
                if "logit" in skip:
                    break
                a = s * slice_n
                m = min(slice_n, n - a)
                # logits for this slice: Wl.T @ l + Wf.T @ f  (PSUM accum)
                lg = ps_log.tile([1, slice_n], f32, tag="logit")
                nc.tensor.matmul(
                    out=lg[:, :m],
                    lhsT=wl_sb[:],
                    rhs=l_t[:, a : a + m],
                    start=True,
                    stop=False,
                )
                nc.tensor.matmul(
                    out=lg[:, :m],
                    lhsT=wf_sb[:],
                    rhs=f_t[:, a : a + m],
                    start=False,
                    stop=True,
                )
                # w = sigmoid(logit + b) on ACT; sole reader of lg
                nc.scalar.activation(
                    out=w_sb[:, a : a + m],
                    in_=lg[:, :m],
                    func=mybir.ActivationFunctionType.Sigmoid,
                    bias=b_sb,
                    scale=1.0,
                )

            # broadcast w across partitions (GPSIMD), then o *= w
            if "bcast" not in skip:
                nc.gpsimd.partition_broadcast(wb_t[:, :n], w_sb[:, :n])
            # finish and store each 1/split slice independently so the output
            # DMA of early slices overlaps the blend tail of later ones
            sw = split if n % split == 0 else 1
            h = n // sw
            no_blend = {"sub", "mul", "add"} <= set(skip)
            for gi, c0 in enumerate(range(0, n, h)):
                if "mul" not in skip:
                    me = mul_engs[gi % len(mul_engs)] if mul_engs else mul_eng
                    eng(me).tensor_mul(
                        out=o_t[:, c0 : c0 + h],
                        in0=o_t[:, c0 : c0 + h],
                        in1=wb_t[:, c0 : c0 + h],
                    )
                if "add" not in skip:
                    # the add carries the int8 conversion (1x on DVE) —
                    # alternate it across engines per split group
                    ae = add_engs[gi % len(add_engs)] if add_engs else add_eng
                    eng(ae).tensor_add(
                        out=o2_t[:, c0 : c0 + h],
                        in0=o_t[:, c0 : c0 + h],
                        in1=f_t[:, c0 : c0 + h],
                    )
                if "out" not in skip and out_split != 1:
                    # if no compute writes o_t (dma-only ablation), store f_t
                    # so the out DMA still moves the same bytes
                    src = f_t if no_blend else o2_t
                    out_dma.dma_start(
                        out=dram_sl(outT, off, c0, h),
                        in_=src[:, c0 : c0 + h],
                    )
            if "out" not in skip and out_split == 1:
                src = f_t if no_blend else o2_t
                out_dma.dma_start(out=dram_sl(outT, off, 0, n), in_=src[:, :n])

    nc.finalize()
    return nc


_NC_CACHE = {}


def _get_nc():
    key = "main"
    if key not in _NC_CACHE:
        _NC_CACHE[key] = build_nc()
    return _NC_CACHE[key]


def make_in_maps(local_embeddings, foreign_embeddings, local_indices, W_att, b_att):
    l_rows = np.ascontiguousarray(local_embeddings[local_indices])  # [M, D]
    if OUT_DTYPE == "i8":
        s = float(
            max(np.abs(l_rows).max(), np.abs(foreign_embeddings).max())
        ) / OUT_HEADROOM
    else:
        s = 1.0
    inv = np.float32(1.0 / s)
    wl = np.ascontiguousarray(W_att[:P].reshape(P, 1) * s, dtype=np.float16)
    wf = np.ascontiguousarray(W_att[P:].reshape(P, 1) * s, dtype=np.float16)
    bbv = np.ascontiguousarray(np.reshape(b_att, (1, 1)), dtype=np.float32)
    in_maps = []
    for i in range(N_CORES):
        sl = slice(i * ROWS_PER_CORE, (i + 1) * ROWS_PER_CORE)
        in_maps.append(
            {
                "lT": np.ascontiguousarray(l_rows[sl].T * inv, dtype=np.float16),
                "fT": np.ascontiguousarray(
                    foreign_embeddings[sl].T * inv, dtype=np.float16
                ),
                "wl": wl,
                "wf": wf,
                "bb": bbv,
            }
        )
    return in_maps, s


def run_device(in_maps, trace=False):
    from concourse.bass_utils import run_bass_kernel_spmd

    return run_bass_kernel_spmd(
        _get_nc(), in_maps, core_ids=list(range(N_CORES)), trace=trace
    )


def kernel(local_embeddings, foreign_embeddings, local_indices, W_att, b_att):
    local_embeddings = np.asarray(local_embeddings, dtype=np.float32)
    foreign_embeddings = np.asarray(foreign_embeddings, dtype=np.float32)
    local_indices = np.asarray(local_indices)
    W_att = np.asarray(W_att, dtype=np.float32)
    b_att = np.asarray(b_att, dtype=np.float32)

    in_maps, s = make_in_maps(
        local_embeddings, foreign_embeddings, local_indices, W_att, b_att
    )
    res = run_device(in_maps)

    updated = np.empty((N_FOREIGN, P), dtype=np.float32)
    for i in range(N_CORES):
        sl = slice(i * ROWS_PER_CORE, (i + 1) * ROWS_PER_CORE)
        updated[sl] = res.results[i]["outT"].T.astype(np.float32) * np.float32(s)

    out = local_embeddings.copy()
    out[local_indices] = updated
    return out
